# revision 5
# baseline (speedup 1.0000x reference)
"""KronyMLP Trainium2 kernel.

Math (per the reference):
    kr1 = kron(c_fc_1 [1536,32], c_fc_2 [1,12])   -> [1536, 384]
    kr2 = kron(c_proj_1 [32,1536], c_proj_2 [12,1]) -> [384, 1536]
    out = gelu_exact(x @ kr1) @ kr2               x: [16, 4096, 1536] f32

Strategy:
  - Host: materialize kr1/kr2 (tiny), shard x data-parallel over batch across
    8 cores (2 batches = 8192 tokens per core), replicate weights.
  - Device (per core): tile tokens in macro-tiles of 512.
      x natural-layout DMA in -> PE transpose (fp32r, via identity) -> x^T in
      SBUF -> MM1 (lhsT=kr1 chunks, rhs=x^T) accumulating over d-chunks into
      PSUM -> exact-erf Gelu on ScalarE (PSUM->SBUF, h^T layout) ->
      MM2 (lhsT=gelu(h^T) token-columns, rhs=kr2) -> PSUM [tokens, d_out]
      natural layout -> copy -> DMA out.
  - All matmuls run in float32r (full fp32 storage; reduced-precision multiply
    at 1 cycle/row for moving dim >= 256 vs 4 cycles/row for exact fp32).
    Set KRONY_MM_DT=f32 to force exact-fp32 matmuls.
"""

import os
import numpy as np

B, S, D = 16, 4096, 1536
H = 384
N_CORES = 8
T_PER_CORE = (B // N_CORES) * S  # 8192
TN = 512  # tokens per macro tile
P = 128

_BUILT = {}


def _build(T, mm_dt_name):
    import concourse.bacc as bacc
    import concourse.mybir as mybir
    from concourse.bass import ts
    from concourse.tile import TileContext

    f32 = mybir.dt.float32
    mm_dt = {"f32r": mybir.dt.float32r, "f32": mybir.dt.float32}[mm_dt_name]

    DC = D // P        # 12 d-model chunks
    HC = H // P        # 3 hidden chunks
    NO = D // 512      # 3 output column chunks
    n_macro = T // TN
    T4 = TN // P       # 4 token sub-tiles per macro

    nc = bacc.Bacc(None, target_bir_lowering=False, debug=False)
    x_d = nc.declare_dram_parameter("x", [T, D], mm_dt, isOutput=False)
    kr1_d = nc.declare_dram_parameter("kr1", [P, DC, H], mm_dt, isOutput=False)
    kr2_d = nc.declare_dram_parameter("kr2", [P, HC, D], mm_dt, isOutput=False)
    id_d = nc.declare_dram_parameter("ident", [P, P], mm_dt, isOutput=False)
    out_d = nc.declare_dram_parameter("out", [T, D], f32, isOutput=True)

    with TileContext(nc) as tc:
        with (
            tc.tile_pool(name="const", bufs=1) as cpool,
            tc.tile_pool(name="xin", bufs=3) as xpool,
            tc.tile_pool(name="xt", bufs=2) as xtpool,
            tc.tile_pool(name="gh", bufs=2) as ghpool,
            tc.tile_pool(name="outp", bufs=3) as opool,
            tc.tile_pool(name="ps_t", bufs=1, space="PSUM") as pst,
            tc.tile_pool(name="ps_h", bufs=1, space="PSUM") as psh,
            tc.tile_pool(name="ps_o", bufs=2, space="PSUM") as pso,
        ):
            ident = cpool.tile([P, P], mm_dt)
            nc.sync.dma_start(out=ident[:], in_=id_d[:, :])
            kr1_sb = cpool.tile([P, DC, H], mm_dt)
            nc.sync.dma_start(out=kr1_sb[:], in_=kr1_d[:, :, :])
            kr2_sb = cpool.tile([P, HC, D], mm_dt)
            nc.sync.dma_start(out=kr2_sb[:], in_=kr2_d[:, :, :])

            for mi in range(n_macro):
                t0 = mi * TN
                # ---- load + transpose x: build x^T [D-chunks, TN] ----
                # One DVE copy per d-chunk writes the exact region MM1 reads,
                # keeping per-matmul sync-wait counts within HW limits.
                xt = xtpool.tile([P, DC, TN], mm_dt)
                xins = []
                for t4 in range(T4):
                    xin = xpool.tile([P, D], mm_dt, tag=f"xin{t4}")
                    nc.sync.dma_start(
                        out=xin[:], in_=x_d[t0 + t4 * P : t0 + (t4 + 1) * P, :]
                    )
                    xins.append(xin)
                for d in range(DC):
                    ps = pst.tile([P, T4, P], mm_dt)
                    for t4 in range(T4):
                        nc.tensor.transpose(
                            ps[:, t4, :], xins[t4][:, ts(d, P)], ident[:]
                        )
                    nc.vector.tensor_copy(out=xt[:, d, :], in_=ps[:])
                # ---- MM1 + gelu: h^T = gelu(kr1^T-chunks @ x^T) ----
                gh = ghpool.tile([P, HC, TN], mm_dt)
                for m in range(HC):
                    ph = psh.tile([P, TN], f32)
                    for d in range(DC):
                        nc.tensor.matmul(
                            ph[:],
                            lhsT=kr1_sb[:, d, ts(m, P)],
                            rhs=xt[:, d, :],
                            start=(d == 0),
                            stop=(d == DC - 1),
                        )
                    nc.scalar.activation(
                        out=gh[:, m, :],
                        in_=ph[:],
                        func=mybir.ActivationFunctionType.Gelu,
                    )
                # ---- MM2: out[tokens, D] = gelu(h)^T-cols @ kr2 ----
                for t4 in range(T4):
                    po = pso.tile([P, NO, 512], f32)
                    for k in range(HC):
                        for n in range(NO):
                            nc.tensor.matmul(
                                po[:, n, :],
                                lhsT=gh[:, k, ts(t4, P)],
                                rhs=kr2_sb[:, k, ts(n, 512)],
                                start=(k == 0),
                                stop=(k == HC - 1),
                            )
                    orow = opool.tile([P, D], f32)
                    for n in range(NO):
                        nc.any.tensor_copy(out=orow[:, ts(n, 512)], in_=po[:, n, :])
                    nc.sync.dma_start(
                        out=out_d[t0 + t4 * P : t0 + (t4 + 1) * P, :], in_=orow[:]
                    )
    nc.finalize()
    return nc


def get_nc(T=T_PER_CORE, mm_dt_name=None):
    if mm_dt_name is None:
        mm_dt_name = os.environ.get("KRONY_MM_DT", "f32r")
    key = (T, mm_dt_name)
    if key not in _BUILT:
        _BUILT[key] = _build(T, mm_dt_name)
    return _BUILT[key]


def _host_weights(c_fc_1, c_fc_2, c_proj_1, c_proj_2):
    kr1 = np.kron(np.asarray(c_fc_1, np.float32), np.asarray(c_fc_2, np.float32))
    kr2 = np.kron(np.asarray(c_proj_1, np.float32), np.asarray(c_proj_2, np.float32))
    # device layouts: kr1 [1536,384] -> [128, 12, 384]; kr2 [384,1536] -> [128, 3, 1536]
    kr1_dev = np.ascontiguousarray(
        kr1.reshape(D // P, P, H).transpose(1, 0, 2)
    )
    kr2_dev = np.ascontiguousarray(
        kr2.reshape(H // P, P, D).transpose(1, 0, 2)
    )
    return kr1_dev, kr2_dev


def run_sharded(x, c_fc_1, c_fc_2, c_proj_1, c_proj_2, T=T_PER_CORE, trace=False,
                tmpdir=None):
    from concourse.bass_utils import run_bass_kernel_spmd

    x = np.asarray(x, np.float32)
    n_tok = x.shape[0] * x.shape[1] * 1  # flattened below
    kr1_dev, kr2_dev = _host_weights(c_fc_1, c_fc_2, c_proj_1, c_proj_2)
    ident = np.eye(P, dtype=np.float32)

    xf = x.reshape(-1, D)
    assert xf.shape[0] == N_CORES * T, (xf.shape, T)
    in_maps = [
        {
            "x": np.ascontiguousarray(xf[i * T : (i + 1) * T]),
            "kr1": kr1_dev,
            "kr2": kr2_dev,
            "ident": ident,
        }
        for i in range(N_CORES)
    ]
    nc = get_nc(T)
    res = run_bass_kernel_spmd(
        nc, in_maps, list(range(N_CORES)), trace=trace, tmpdir=tmpdir
    )
    outs = [res.results[i]["out"] for i in range(N_CORES)]
    full = np.concatenate(outs, axis=0).reshape(x.shape)
    return full, res


def kernel(x, c_fc_1, c_fc_2, c_proj_1, c_proj_2):
    out, _ = run_sharded(x, c_fc_1, c_fc_2, c_proj_1, c_proj_2)
    return out.astype(np.float32)


# revision 6
# speedup vs baseline: 2.7684x; 2.7684x over previous
"""KronyMLP Trainium2 kernel.

Math (per the reference):
    kr1 = kron(c_fc_1 [1536,32], c_fc_2 [1,12])   -> [1536, 384]
    kr2 = kron(c_proj_1 [32,1536], c_proj_2 [12,1]) -> [384, 1536]
    out = gelu_exact(x @ kr1) @ kr2               x: [16, 4096, 1536] f32

Strategy:
  - Host: materialize kr1/kr2 (tiny), shard x data-parallel over batch across
    8 cores (2 batches = 8192 tokens per core), replicate weights.
  - Device (per core): tile tokens in macro-tiles of 512.
      x natural-layout DMA in -> PE transpose (fp32r, via identity) -> x^T in
      SBUF -> MM1 (lhsT=kr1 chunks, rhs=x^T) accumulating over d-chunks into
      PSUM -> exact-erf Gelu on ScalarE (PSUM->SBUF, h^T layout) ->
      MM2 (lhsT=gelu(h^T) token-columns, rhs=kr2) -> PSUM [tokens, d_out]
      natural layout -> copy -> DMA out.
  - All matmuls run in float32r (full fp32 storage; reduced-precision multiply
    at 1 cycle/row for moving dim >= 256 vs 4 cycles/row for exact fp32).
    Set KRONY_MM_DT=f32 to force exact-fp32 matmuls.
"""

import os
import numpy as np

B, S, D = 16, 4096, 1536
H = 384
N_CORES = 8
T_PER_CORE = (B // N_CORES) * S  # 8192
TN = 512  # tokens per macro tile
P = 128

_BUILT = {}


def _build(T, mm_dt_name):
    import concourse.bacc as bacc
    import concourse.mybir as mybir
    from concourse.bass import ts
    from concourse.tile import TileContext

    f32 = mybir.dt.float32
    mm_dt = {"f32r": mybir.dt.float32r, "f32": mybir.dt.float32}[mm_dt_name]

    DC = D // P        # 12 d-model chunks
    HC = H // P        # 3 hidden chunks
    NO = D // 512      # 3 output column chunks
    n_macro = T // TN
    T4 = TN // P       # 4 token sub-tiles per macro

    nc = bacc.Bacc(None, target_bir_lowering=False, debug=False)
    x_d = nc.declare_dram_parameter("x", [T, D], mm_dt, isOutput=False)
    kr1_d = nc.declare_dram_parameter("kr1", [P, DC, H], mm_dt, isOutput=False)
    kr2_d = nc.declare_dram_parameter("kr2", [P, HC, D], mm_dt, isOutput=False)
    id_d = nc.declare_dram_parameter("ident", [P, P], mm_dt, isOutput=False)
    out_d = nc.declare_dram_parameter("out", [T, D], f32, isOutput=True)

    with TileContext(nc) as tc:
        with (
            tc.tile_pool(name="const", bufs=1) as cpool,
            tc.tile_pool(name="xin", bufs=3) as xpool,
            tc.tile_pool(name="xt", bufs=2) as xtpool,
            tc.tile_pool(name="gh", bufs=2) as ghpool,
            tc.tile_pool(name="outp", bufs=3) as opool,
            tc.tile_pool(name="ps_t", bufs=1, space="PSUM") as pst,
            tc.tile_pool(name="ps_h", bufs=1, space="PSUM") as psh,
            tc.tile_pool(name="ps_o", bufs=2, space="PSUM") as pso,
        ):
            ident = cpool.tile([P, P], mm_dt)
            nc.sync.dma_start(out=ident[:], in_=id_d[:, :])
            kr1_sb = cpool.tile([P, DC, H], mm_dt)
            nc.sync.dma_start(out=kr1_sb[:], in_=kr1_d[:, :, :])
            kr2_sb = cpool.tile([P, HC, D], mm_dt)
            nc.sync.dma_start(out=kr2_sb[:], in_=kr2_d[:, :, :])

            for mi in range(n_macro):
                t0 = mi * TN
                # ---- load + transpose x: build x^T [D-chunks, TN] ----
                # One DVE copy per d-chunk writes the exact region MM1 reads,
                # keeping per-matmul sync-wait counts within HW limits.
                xt = xtpool.tile([P, DC, TN], mm_dt)
                xins = []
                for t4 in range(T4):
                    xin = xpool.tile([P, D], mm_dt, tag=f"xin{t4}")
                    nc.sync.dma_start(
                        out=xin[:], in_=x_d[t0 + t4 * P : t0 + (t4 + 1) * P, :]
                    )
                    xins.append(xin)
                for d in range(DC):
                    ps = pst.tile([P, T4, P], mm_dt)
                    for t4 in range(T4):
                        nc.tensor.transpose(
                            ps[:, t4, :], xins[t4][:, ts(d, P)], ident[:]
                        )
                    nc.vector.tensor_copy(out=xt[:, d, :], in_=ps[:])
                # ---- MM1 + gelu: h^T = gelu(kr1^T-chunks @ x^T) ----
                gh = ghpool.tile([P, HC, TN], mm_dt)
                for m in range(HC):
                    ph = psh.tile([P, TN], f32)
                    for d in range(DC):
                        nc.tensor.matmul(
                            ph[:],
                            lhsT=kr1_sb[:, d, ts(m, P)],
                            rhs=xt[:, d, :],
                            start=(d == 0),
                            stop=(d == DC - 1),
                        )
                    nc.scalar.activation(
                        out=gh[:, m, :],
                        in_=ph[:],
                        func=mybir.ActivationFunctionType.Gelu,
                    )
                # ---- MM2: out[tokens, D] = gelu(h)^T-cols @ kr2 ----
                for t4 in range(T4):
                    po = pso.tile([P, NO, 512], f32)
                    for k in range(HC):
                        for n in range(NO):
                            nc.tensor.matmul(
                                po[:, n, :],
                                lhsT=gh[:, k, ts(t4, P)],
                                rhs=kr2_sb[:, k, ts(n, 512)],
                                start=(k == 0),
                                stop=(k == HC - 1),
                            )
                    orow = opool.tile([P, D], f32)
                    for n in range(NO):
                        nc.vector.tensor_copy(out=orow[:, ts(n, 512)], in_=po[:, n, :])
                    nc.sync.dma_start(
                        out=out_d[t0 + t4 * P : t0 + (t4 + 1) * P, :], in_=orow[:]
                    )
    nc.finalize()
    return nc


def get_nc(T=T_PER_CORE, mm_dt_name=None):
    if mm_dt_name is None:
        mm_dt_name = os.environ.get("KRONY_MM_DT", "f32r")
    key = (T, mm_dt_name)
    if key not in _BUILT:
        _BUILT[key] = _build(T, mm_dt_name)
    return _BUILT[key]


def _host_weights(c_fc_1, c_fc_2, c_proj_1, c_proj_2):
    kr1 = np.kron(np.asarray(c_fc_1, np.float32), np.asarray(c_fc_2, np.float32))
    kr2 = np.kron(np.asarray(c_proj_1, np.float32), np.asarray(c_proj_2, np.float32))
    # device layouts: kr1 [1536,384] -> [128, 12, 384]; kr2 [384,1536] -> [128, 3, 1536]
    kr1_dev = np.ascontiguousarray(
        kr1.reshape(D // P, P, H).transpose(1, 0, 2)
    )
    kr2_dev = np.ascontiguousarray(
        kr2.reshape(H // P, P, D).transpose(1, 0, 2)
    )
    return kr1_dev, kr2_dev


def run_sharded(x, c_fc_1, c_fc_2, c_proj_1, c_proj_2, T=T_PER_CORE, trace=False,
                tmpdir=None):
    from concourse.bass_utils import run_bass_kernel_spmd

    x = np.asarray(x, np.float32)
    n_tok = x.shape[0] * x.shape[1] * 1  # flattened below
    kr1_dev, kr2_dev = _host_weights(c_fc_1, c_fc_2, c_proj_1, c_proj_2)
    ident = np.eye(P, dtype=np.float32)

    xf = x.reshape(-1, D)
    assert xf.shape[0] == N_CORES * T, (xf.shape, T)
    in_maps = [
        {
            "x": np.ascontiguousarray(xf[i * T : (i + 1) * T]),
            "kr1": kr1_dev,
            "kr2": kr2_dev,
            "ident": ident,
        }
        for i in range(N_CORES)
    ]
    nc = get_nc(T)
    res = run_bass_kernel_spmd(
        nc, in_maps, list(range(N_CORES)), trace=trace, tmpdir=tmpdir
    )
    outs = [res.results[i]["out"] for i in range(N_CORES)]
    full = np.concatenate(outs, axis=0).reshape(x.shape)
    return full, res


def kernel(x, c_fc_1, c_fc_2, c_proj_1, c_proj_2):
    out, _ = run_sharded(x, c_fc_1, c_fc_2, c_proj_1, c_proj_2)
    return out.astype(np.float32)
